# revision 5
# baseline (speedup 1.0000x reference)
"""Lorenz Euler integration on Trainium2 (Bass/Tile).

Algorithm: the Euler recurrence
    x' = (1-dt*s)*x + dt*s*y
    y' = (1-dt)*y   + dt*x*(r-z)
    z' = (1-dt*b)*z + dt*x*y
is solved by global Gauss-Seidel sweeps over the whole trajectory. Each
component, given the others, is an exact constant-coefficient linear
recurrence solved in parallel with a blocked scan:
  phase1: per-chunk tensor_tensor_scan (state = a*state + f, init 0)
  phase2: chunk-boundary states E = T @ q via one PE matmul with a
          host-precomputed Toeplitz decay matrix (plus an s0 column)
  phase3: states = a^i * E + partial (scalar_tensor_tensor)
~30 sweeps reach the fp32 rounding floor (~7e-6 rel err vs the
sequential fp32 reference).

Layout: 3999 transitions in C=125 chunks x L=32 (one extra state 4000,
discarded). buf_k[125, 33]: col 0 = chunk-start state, cols 1..32 =
chunk states. All parameters (sigma, rho, beta, stats) are baked into
immediates / host-built constant tables at trace time.
"""
import sys
import numpy as np

sys.path.insert(0, "/opt/trn_rl_repo")

N = 4000
C = 125          # chunks (partitions)
L = 32           # steps per chunk
DT = 0.01
SWEEPS = 30
N_CORES = 8


def _build_consts(a_vals, s0_vals):
    """Host-precomputed fp32 tables: per component k the decay powers
    apow[c,i] = a^(i+1) and the chunk-propagation matrix lhsT with
    E = T @ qaug, T[c,j] = (a^L)^(c-1-j) for j<c, T[c,125] = a^(L*c)."""
    apows = np.zeros((3 * C, L), np.float32)
    tmats = np.zeros((3 * 128, 128), np.float32)
    for k, a in enumerate(a_vals):
        a = np.float64(a)
        apows[k * C:(k + 1) * C, :] = (a ** np.arange(1, L + 1))[None, :]
        aL = a ** L
        T = np.zeros((128, 128), np.float64)
        for c in range(C):
            j = np.arange(0, c)
            T[c, j] = aL ** (c - 1 - j)
            T[c, 125] = a ** (L * c)
        tmats[k * 128:(k + 1) * 128, :] = T.T.astype(np.float32)
    return apows, tmats


def _build_module(sigma, rho, beta, stats):
    import concourse.bass as bass
    import concourse.tile as tile
    import concourse.mybir as mybir
    from concourse import bacc

    FP32 = mybir.dt.float32
    mult = mybir.AluOpType.mult
    add = mybir.AluOpType.add
    sub = mybir.AluOpType.subtract

    a_vals = [1.0 - DT * sigma, 1.0 - DT, 1.0 - DT * beta]   # x, y, z
    # scaled state: xhat = -dt*x; y,z plain. Every forcing is one DVE op:
    #   f_y = (z-rho)*xhat = dt*x*(rho-z)            (y-scan: a*s + f)
    #   xhat*y = -dt*x*y -> z-scan uses op1=subtract (a*s - f)
    #   f_xhat = -dt^2*sigma*y                        (x-scan: a*s + f)
    s0 = [float(-DT * stats[0]), float(stats[1]), float(stats[2])]

    nc = bacc.Bacc("TRN2", target_bir_lowering=False)
    stats_col = nc.dram_tensor("stats_col", [3, 1], FP32, kind="ExternalInput")
    stats_row = nc.dram_tensor("stats_row", [1, 3], FP32, kind="ExternalInput")
    tmats_in = nc.dram_tensor("tmats", [3 * 128, 128], FP32, kind="ExternalInput")
    apows_in = nc.dram_tensor("apows", [3 * C, L], FP32, kind="ExternalInput")
    out_h = nc.dram_tensor("out", [N * 3], FP32, kind="ExternalOutput")

    with tile.TileContext(nc) as tc:
        with tc.tile_pool(name="sb", bufs=1) as pool, \
             tc.tile_pool(name="ps", bufs=1, space="PSUM") as psum:
            lhsT = [pool.tile([128, 128], FP32, tag=f"lhsT{k}", name=f"lhsT{k}") for k in range(3)]
            apow = [pool.tile([C, L], FP32, tag=f"apow{k}", name=f"apow{k}") for k in range(3)]
            acst = [pool.tile([C, L], FP32, tag=f"acst{k}", name=f"acst{k}") for k in range(3)]
            buf = [pool.tile([C, L + 1], FP32, tag=f"buf{k}", name=f"buf{k}") for k in range(3)]
            part = [pool.tile([128, L], FP32, tag=f"part{k}", name=f"part{k}") for k in range(3)]
            forc = [pool.tile([C, L], FP32, tag=f"forc{k}", name=f"forc{k}") for k in range(3)]
            e_ps = [psum.tile([128, 1], FP32, tag=f"eps{k}", name=f"eps{k}") for k in range(3)]
            st_row = pool.tile([1, 3], FP32, tag="strow")
            staging = pool.tile([C, 3 * L], FP32, tag="staging")

            # ---- init ----
            for k in range(3):
                nc.gpsimd.dma_start(lhsT[k][:], tmats_in[k * 128:(k + 1) * 128, :])
                nc.gpsimd.dma_start(apow[k][:], apows_in[k * C:(k + 1) * C, :])
                nc.vector.memset(acst[k][:], float(a_vals[k]))
                nc.vector.memset(buf[k][:], s0[k])
                nc.vector.memset(part[k][:], 0.0)
                # s0 feeds the matmul via qaug row 125
                nc.gpsimd.dma_start(part[k][125:126, L - 1:L], stats_col[k:k + 1, 0:1])
            nc.gpsimd.dma_start(st_row[:], stats_row[:])

            X, Y, Z = 0, 1, 2

            def solve(k, op1=add):
                """phase1 scan -> phase2 matmul -> E copy -> phase3."""
                nc.vector.tensor_tensor_scan(
                    part[k][0:C, :], acst[k][:], forc[k][:], 0.0, mult, op1)
                nc.tensor.matmul(e_ps[k][:], lhsT[k][:], part[k][:, L - 1:L],
                                 start=True, stop=True)
                nc.scalar.copy(buf[k][:, 0:1], e_ps[k][0:C, :])
                nc.vector.scalar_tensor_tensor(
                    buf[k][:, 1:L + 1], apow[k][:], e_ps[k][0:C, 0:1],
                    part[k][0:C, :], mult, add)

            for _ in range(SWEEPS):
                nc.vector.scalar_tensor_tensor(
                    forc[Y][:], buf[Z][:, 0:L], float(rho), buf[X][:, 0:L],
                    sub, mult)
                solve(Y)
                nc.vector.tensor_tensor(forc[Z][:], buf[X][:, 0:L],
                                        buf[Y][:, 0:L], mult)
                solve(Z, op1=sub)
                nc.vector.tensor_scalar_mul(forc[X][:], buf[Y][:, 0:L],
                                            float(-DT * DT * sigma))
                solve(X)

            # ---- output assembly: interleave x,y,z then DMA ----
            unscale = [-1.0 / DT, 1.0, 1.0]
            for k in range(3):
                nc.vector.tensor_scalar_mul(
                    staging[:].rearrange("c (i three) -> c i three", three=3)[:, :, k],
                    buf[k][:, 1:L + 1], unscale[k])
            nc.gpsimd.dma_start(out_h[0:3].rearrange("(a b) -> a b", a=1), st_row[:])
            nc.gpsimd.dma_start(
                out_h[3:3 + 124 * 96].rearrange("(c f) -> c f", f=96),
                staging[0:124, :])
            nc.gpsimd.dma_start(
                out_h[3 + 124 * 96:N * 3].rearrange("(a b) -> a b", a=1),
                staging[124:125, 0:93])

    nc.compile()
    return nc


def kernel(t, sigma, rho, beta, stats):
    from concourse.bass_utils import run_bass_kernel_spmd

    sigma = float(np.asarray(sigma).reshape(-1)[0])
    rho = float(np.asarray(rho).reshape(-1)[0])
    beta = float(np.asarray(beta).reshape(-1)[0])
    stats = np.asarray(stats, np.float32).reshape(3)

    a_vals = [1.0 - DT * sigma, 1.0 - DT, 1.0 - DT * beta]
    apows, tmats = _build_consts(a_vals, stats)
    nc = _build_module(sigma, rho, beta, stats)

    stats_scaled = np.array([-DT * stats[0], stats[1], stats[2]], np.float32)
    in_map = {
        "stats_col": stats_scaled.reshape(3, 1).copy(),
        "stats_row": stats.reshape(1, 3).copy(),
        "tmats": tmats,
        "apows": apows,
    }
    import os
    trace = bool(int(os.environ.get("LORENZ_TRACE", "0")))
    res = run_bass_kernel_spmd(nc, [dict(in_map) for _ in range(N_CORES)],
                               core_ids=list(range(N_CORES)), trace=trace)
    if trace and res.exec_time_ns is not None:
        print(f"HW exec time: {res.exec_time_ns} ns")
        if res.instructions_and_trace is not None:
            print("trace:", res.instructions_and_trace[1])
    out = res.results[0]["out"].reshape(N, 3).astype(np.float32)
    return out


if __name__ == "__main__":
    t = np.arange(0, 40, 0.01, dtype=np.float32)
    one = np.ones(1, np.float32)
    out = kernel(t=t, sigma=one, rho=one, beta=one, stats=np.ones(3, np.float32))
    print(out[:3], out[-2:])
